# revision 25
# baseline (speedup 1.0000x reference)
"""Causal multi-head attention (B=2, S=2048, E=1024, H=16) on 8 TRN2 NeuronCores.

Sharding: 8 cores = 2 batches x 4 head-groups (4 heads / 256 dims each).
Each core loads its batch's q/k/v (pre-transposed to [E, S] on host), its
head-group's Wq/Wk/Wv column-slices and Wo row-slice, computes projections +
causal attention + a partial output projection [S, E]; the host sums the 4
partials per batch and adds the bias.

Streaming data is fp16 (eps ~5e-4; halves DMA); PSUM accumulation is fp32.
Softmax is computed max-free (exp(s/8) cannot overflow for this data scale)
in the transposed [t, s] domain.

Per t-tile the two heads of each pair occupy disjoint K=64 PE row groups so
their score matmuls pack (concurrent row-group execution, distinct PSUM
banks; verified on HW ~2x). The score tile [128, 2heads, 512] is
double-buffered (2x2 banks) and the attn@V matmuls trail one tile behind, so
the PE emits scores(t+1) + filler work while the Act engine exps tile t:
Act stays saturated and the PE never stalls on the exp latency. The causal
mask costs only a 128-wide diagonal band: one K=128 identity-matmul of a
lower-triangular -30000 pattern per head bank (start of the band group),
band + off-band score matmuls split so fully-masked columns are never
computed; the exp APs narrow past them too. The softmax denominator rides
the attn@V matmul via a ones-column appended to V; normalization is an fp16
reciprocal + K=1 ones-matmul broadcast into the unused upper half of the
attn PSUM tile + one multiply.

The attention phase is Activation-bound, so all other PE work rides inside
it as filler: Q/K/V projection chunks of the next s-block (rotating into the
next repeat at the last block) and floating output-projection chunks,
weighted toward the late (long) blocks. Do NOT let two matmuls accumulate
into the same PSUM region from different PE row groups (device hang), and
keep a consistent row-group->bank mapping for packed pairs; GPSIMD cannot
access PSUM.
"""

import numpy as np

B, S, E, H, D = 2, 2048, 1024, 16, 64
HPC = 4              # heads per core
L = HPC * D          # 256 local dims per core
SB = 512             # s-block (softmax/matmul free-dim block)
NSB = S // SB        # 4
TT = 128             # t-tile
NTT = S // TT        # 16
NET = E // 128       # 8 contraction tiles over E

_cache = {}


def _patch_tile_drain():
    """This container's walrus allows only one sync-wait per instruction.
    Split the TileContext tail-drain waits across standalone SP nops."""
    import bass_rust
    import concourse.tile as tile
    from concourse.vector_clock import ScopedClock

    if getattr(tile.TileContext, "_drain_patched", False):
        return

    def _drain_and_barrier(self, tick_clock, wait_clock):
        drain_inst = self.nc.sync.drain()
        wait_clock.add_sem_waits(
            drain_inst.ins, ScopedClock({None: tick_clock.global_clock})
        )
        si = drain_inst.ins.sync_info
        if si is not None and len(si.on_wait) > 1:
            waits = list(si.on_wait)
            drain_inst.ins.sync_info = bass_rust.SyncInfo(
                on_wait=[waits[0]], on_update=list(si.on_update)
            )
            for w in waits[1:]:
                nop = self.nc.sync.nop(nofuse=True)
                nop.ins.sync_info = bass_rust.SyncInfo(on_wait=[w], on_update=[])
        self.nc.all_engine_barrier()
        assert self.sems is not None
        popped = self.nc._tile_sem_poison_stack.pop()
        assert popped is self._sem_poison
        self.nc.clear_and_free_semaphores(list(self.sems.allocated().values()))
        self.nc.all_engine_barrier()

    tile.TileContext._drain_and_barrier = _drain_and_barrier
    tile.TileContext._drain_patched = True


def _split_multi_waits(nc):
    """Move extra per-instruction semaphore waits onto standalone same-engine
    NoOps inserted immediately before the instruction (walrus 1-wait limit)."""
    import bass_rust

    def make_wait_nop(engine_ty, wait):
        eng = None
        for e in (nc.tensor, nc.scalar, nc.vector, nc.gpsimd, nc.sync):
            if e.engine == engine_ty:
                eng = e
                break
        assert eng is not None, f"no engine object for {engine_ty}"
        bi = eng.nop(nofuse=True)
        inst = bi.ins
        bb = nc.cur_bb.bb if nc.cur_bb is not None else None
        if bb is not None and bb.instructions and bb.instructions[-1] is inst:
            bb.instructions.pop()
        inst.sync_info = bass_rust.SyncInfo(on_wait=[wait], on_update=[])
        return inst

    f = nc.m.functions[0]
    for blk in f.blocks:
        new_list = []
        changed = False
        for inst in blk.instructions:
            si = inst.sync_info
            if si is not None and len(si.on_wait) > 1:
                waits = list(si.on_wait)
                for w in waits[:-1]:
                    new_list.append(make_wait_nop(inst.engine, w))
                inst.sync_info = bass_rust.SyncInfo(
                    on_wait=[waits[-1]], on_update=list(si.on_update)
                )
                changed = True
            new_list.append(inst)
        if changed:
            blk.instructions = new_list


def _build(repeat=1):
    import concourse.bass as bass
    import concourse.tile as tile
    from concourse import mybir

    _patch_tile_drain()

    f32 = mybir.dt.float32
    f16 = mybir.dt.float16
    EXP = mybir.ActivationFunctionType.Exp
    COPY = mybir.ActivationFunctionType.Copy
    MULT = mybir.AluOpType.mult

    nc = bass.Bass()
    qT = nc.declare_dram_parameter("qT", [E, S], f16, isOutput=False)
    kT = nc.declare_dram_parameter("kT", [E, S], f16, isOutput=False)
    vT = nc.declare_dram_parameter("vT", [E, S], f16, isOutput=False)
    wq = nc.declare_dram_parameter("wq", [E, L], f16, isOutput=False)
    wk = nc.declare_dram_parameter("wk", [E, L], f16, isOutput=False)
    wv = nc.declare_dram_parameter("wv", [E, L], f16, isOutput=False)
    wo = nc.declare_dram_parameter("wo", [L, E], f16, isOutput=False)
    ident = nc.declare_dram_parameter("ident", [128, 128], f16, isOutput=False)
    bnb = nc.declare_dram_parameter("bnb", [128, 128], f16, isOutput=False)
    out = nc.declare_dram_parameter("out", [S, E], f16, isOutput=True)

    with tile.TileContext(nc) as tc:
        with (
            tc.tile_pool(name="const", bufs=1) as const,
            tc.tile_pool(name="resid", bufs=1) as resid,
            tc.tile_pool(name="qk_in", bufs=4) as qk_in,
            tc.tile_pool(name="v_in", bufs=3) as v_in,
            tc.tile_pool(name="expp", bufs=5) as expp,
            tc.tile_pool(name="sm", bufs=3) as smp,
            tc.tile_pool(name="outp", bufs=6) as outp,
        ):
            # ---- resident weights / masks ----
            wq_sb = const.tile([128, NET, L], f16, tag="wq")
            wk_sb = const.tile([128, NET, L], f16, tag="wk")
            wv_sb = const.tile([128, NET, L], f16, tag="wv")
            wo_sb = const.tile([128, 2, E], f16, tag="wo")
            id_sb = const.tile([128, 128], f16, tag="ident")
            bn_sb = const.tile([128, 128], f16, tag="bnb")
            ones_f32 = const.tile([128, 1], f32, tag="ones32")
            ones_r = const.tile([1, 64], f16, tag="onesr")
            # ordered by first use (wq/wk for the prologue, ident/bnb for
            # block-0 diag tiles, wv for the block-0 v-chunk fillers, wo only
            # at the first oproj filler). These ride the Pool/SWDGE path so
            # they don't contend with the streaming input DMAs on HWDGE.
            nc.gpsimd.dma_start(id_sb[:], ident[:])
            nc.gpsimd.dma_start(bn_sb[:], bnb[:])
            nc.gpsimd.dma_start(wv_sb[:], wv.rearrange("(n p) l -> p n l", p=128))
            nc.gpsimd.dma_start(wo_sb[:], wo.rearrange("(h p) e -> p h e", p=128))
            nc.any.memset(ones_f32[:], 1.0)
            nc.vector.tensor_copy(
                ones_r[:], ones_f32[0:1, :].broadcast_to([1, 64])
            )

            # ---- residents ----
            QT = resid.tile([128, 2, S], f16, tag="QT")   # [dim%128, dimpair, s]
            KT = resid.tile([128, 2, S], f16, tag="KT")
            Vn = resid.tile([128, NTT, HPC, 65], f16, tag="Vn")  # V nat + ones col
            PT = resid.tile([128, 2, S], f16, tag="PT")   # normalized attn out.T
            nc.vector.tensor_copy(
                Vn[:, :, :, 64:65],
                ones_f32[:, None, None, :].broadcast_to([128, NTT, HPC, 1]),
            )

            with (
                tc.tile_pool(name="ps_pp", bufs=1, space="PSUM") as ps_pp,
                tc.tile_pool(name="ps_o", bufs=1, space="PSUM") as ps_o,
                tc.tile_pool(name="ps_sc", bufs=2, space="PSUM") as ps_sc,
                tc.tile_pool(name="ps_av", bufs=1, space="PSUM") as ps_av,
            ):
                if True:
                    # stream tiles for the s-block currently being projected
                    xt_tiles = {}


                    def issue_qk_dma(sbn, split=False):
                        # one DMA instruction per tensor per block: HWDGE
                        # descriptor generation (~625ns/instr, serialized
                        # device-wide) is the scarce resource, not bandwidth.
                        # Block 0 splits per k-tile instead so the prologue's
                        # first matmuls start after ~128KB, and the weight
                        # DMAs interleave rather than queue behind 3MB.
                        for nm, x_dram in (("q", qT), ("k", kT)):
                            xt = qk_in.tile([128, NET, SB], f16, tag=nm + "t")
                            src = x_dram.rearrange("(n p) s -> p n s", p=128)[
                                :, :, sbn * SB : (sbn + 1) * SB
                            ]
                            if split:
                                for kk in range(NET):
                                    nc.sync.dma_start(xt[:, kk, :], src[:, kk, :])
                            else:
                                nc.sync.dma_start(xt[:], src)
                            xt_tiles[(nm, sbn)] = xt

                    def issue_v_dma(sbn):
                        vt = v_in.tile([128, NET, SB], f16, tag="vt")
                        nc.sync.dma_start(
                            vt[:],
                            vT.rearrange("(n p) s -> p n s", p=128)[
                                :, :, sbn * SB : (sbn + 1) * SB
                            ],
                        )
                        xt_tiles[("v", sbn)] = vt

                    def psum_tile(pool):
                        tag = "po" if pool is ps_o else "pp"
                        return pool.tile([128, SB], f32, tag=tag, name=tag)

                    def qk_chunk(sbn, nm, hp, pool=None):
                        w_sb = wq_sb if nm == "q" else wk_sb
                        dst = QT if nm == "q" else KT
                        xt = xt_tiles[(nm, sbn)]
                        ps = psum_tile(pool if pool is not None else ps_pp)
                        for kk in range(NET):
                            nc.tensor.matmul(
                                ps[:],
                                w_sb[:, kk, hp * 128 : (hp + 1) * 128],
                                xt[:, kk, :],
                                start=(kk == 0),
                                stop=(kk == NET - 1),
                            )
                        nc.vector.tensor_copy(
                            dst[:, hp, sbn * SB : (sbn + 1) * SB], ps[:]
                        )

                    def v_chunk(ttn):
                        vt = xt_tiles[("v", ttn // 4)]
                        tl = (ttn % 4) * TT
                        psf = psum_tile(ps_pp)
                        ps = psf[:, 0:L]
                        for kk in range(NET):
                            nc.tensor.matmul(
                                ps[:],
                                vt[:, kk, tl : tl + TT],
                                wv_sb[:, kk, :],
                                start=(kk == 0),
                                stop=(kk == NET - 1),
                            )
                        nc.vector.tensor_copy(
                            Vn[:, ttn, :, 0:64],
                            ps[:].rearrange("p (h d) -> p h d", d=64),
                        )

                    def oproj_chunk(st, eb, pool=None, eng="vector",
                                    dma="gpsimd"):
                        pso = psum_tile(pool if pool is not None else ps_o)
                        for hp in range(2):
                            nc.tensor.matmul(
                                pso[:],
                                PT[:, hp, st * 128 : (st + 1) * 128],
                                wo_sb[:, hp, eb * SB : (eb + 1) * SB],
                                start=(hp == 0),
                                stop=(hp == 1),
                            )
                        ot = outp.tile([128, SB], f16, tag="ot")
                        if eng == "scalar":
                            # epilogue only: Act is idle there and Copy shares
                            # the exp table set (no table reload)
                            nc.scalar.activation(ot[:], pso[:], COPY, scale=1.0)
                        else:
                            nc.vector.tensor_copy(ot[:], pso[:])
                        # mid-stream writes ride Pool/SWDGE (idle after the
                        # consts) to keep HWDGE free for input streaming; the
                        # epilogue uses HWDGE (idle by then)
                        dma_eng = nc.gpsimd if dma == "gpsimd" else nc.sync
                        dma_eng.dma_start(
                            out[
                                st * 128 : (st + 1) * 128,
                                eb * SB : (eb + 1) * SB,
                            ],
                            ot[:],
                        )

                    # ---- prologue: q/k projections of block 0 only; the
                    # block-0 v-chunks ride as block-0 attention fillers.
                    # DMAs for blocks 0 AND 1 are issued up front (2-block
                    # lookahead continues in the loop) so proj fillers never
                    # catch up with their input DMA. qk chunks alternate
                    # ps_pp/ps_o (ps_o is idle until the first oproj) to
                    # dodge the single-bank WAR stall. ----
                    NBLK = repeat * NSB

                    def issue_block(g, split=False):
                        if g < NBLK:
                            issue_qk_dma(g % NSB, split=split)
                            issue_v_dma(g % NSB)

                    # startup order on the sync/HWDGE queue: wq, q0 (per
                    # k-tile), wk, k0 (per k-tile) — each weight arrives just
                    # before the chunks that need it, and the split q0/k0
                    # pieces drip in under the running matmuls
                    nc.sync.dma_start(
                        wq_sb[:], wq.rearrange("(n p) l -> p n l", p=128)
                    )
                    q0 = qk_in.tile([128, NET, SB], f16, tag="qt")
                    for kk in range(NET):
                        nc.sync.dma_start(
                            q0[:, kk, :],
                            qT.rearrange("(n p) s -> p n s", p=128)[:, kk, 0:SB],
                        )
                    xt_tiles[("q", 0)] = q0
                    nc.sync.dma_start(
                        wk_sb[:], wk.rearrange("(n p) l -> p n l", p=128)
                    )
                    k0 = qk_in.tile([128, NET, SB], f16, tag="kt")
                    for kk in range(NET):
                        nc.sync.dma_start(
                            k0[:, kk, :],
                            kT.rearrange("(n p) s -> p n s", p=128)[:, kk, 0:SB],
                        )
                    xt_tiles[("k", 0)] = k0
                    issue_v_dma(0)
                    issue_block(1)
                    for i, (nm, hp) in enumerate(
                        (n, h) for n in ("q", "k") for h in range(2)
                    ):
                        qk_chunk(0, nm, hp, pool=(ps_pp, ps_o)[i % 2])

                    # ---- steady-state pipeline, rotated one block across
                    # repeats: during attention(sb) the PE filler work is the
                    # projection of block (sb+1) mod 4 (of the next repeat
                    # when sb==3) and the output projection of block (sb-1)
                    # mod 4 (of the previous repeat when sb==0). ----
                    for rep, sb in ((r, b) for r in range(repeat)
                                    for b in range(NSB)):
                        g = rep * NSB + sb
                        issue_block(g + 2)
                        sbs = slice(sb * SB, (sb + 1) * SB)

                        proj_f, op_f = [], []
                        if g == 0:
                            for ttn in range(4):
                                proj_f.append((v_chunk, (ttn,)))
                        if g + 1 < NBLK:
                            nsb = (sb + 1) % NSB
                            for nm in ("q", "k"):
                                for hp in range(2):
                                    proj_f.append((qk_chunk, (nsb, nm, hp)))
                            for ttn in range(4 * nsb, 4 * (nsb + 1)):
                                proj_f.append((v_chunk, (ttn,)))
                        # out-projection chunks float: weight them toward the
                        # late (long-attention) blocks so the PE filler supply
                        # roughly tracks each block's g-visit count.
                        # sb0: outproj(3) of the previous repeat (forced);
                        # sb1: first half of outproj(0); sb2: rest of
                        # outproj(0) + outproj(1); sb3: outproj(2).
                        def op_chunks(blk, lo, hi):
                            return [
                                (oproj_chunk, (st, eb))
                                for st in range(4 * blk, 4 * (blk + 1))
                                for eb in range(E // SB)
                            ][lo:hi]
                        if sb == 0 and rep > 0:
                            op_f += op_chunks(3, 0, 8)
                        elif sb == 2:
                            op_f += op_chunks(0, 0, 8)
                        elif sb == 3:
                            op_f += op_chunks(1, 0, 8) + op_chunks(2, 0, 8)
                        fillers = []
                        while op_f or proj_f:
                            if op_f:
                                fillers.append(op_f.pop(0))
                            if proj_f:
                                fillers.append(proj_f.pop(0))

                        n_tt = (sb + 1) * (SB // TT)  # causal t-tiles
                        n_vis = 2 * n_tt              # tile-visits over both hp
                        fill_i = 0
                        vis_i = 0

                        def emit_fillers():
                            nonlocal fill_i, vis_i
                            vis_i += 1
                            rem_vis = n_vis - vis_i + 1
                            want = -((len(fillers) - fill_i) // -rem_vis)
                            for _ in range(want):
                                if fill_i < len(fillers):
                                    fn, a = fillers[fill_i]
                                    fn(*a)
                                    fill_i += 1

                        # ---- attention for this s-block: per t-tile, head
                        # pair packed into disjoint K=64 PE row groups and the
                        # two PSUM banks of a double-buffered score tile; the
                        # attn@V matmuls trail one tile behind so the PE can
                        # emit scores(t+1) + filler while the Act engine exps
                        # tile t -- Act stays saturated, PE never idles. ----
                        for hp in range(2):
                            av0 = ps_av.tile([128, SB], f32, tag="av0")
                            av1 = ps_av.tile([128, SB], f32, tag="av1")
                            pend = []  # [(tt, et, no), ...] av trails 2 back
                            for idx in range(n_tt):
                                tt = idx
                                tts = slice(tt * TT, (tt + 1) * TT)
                                diag = tt >= sb * 4
                                sc = ps_sc.tile([128, 2, SB], f32, tag="sc")
                                if diag:
                                    # causal: -30000 lower-tri pattern in the
                                    # 128-wide diagonal band via identity
                                    # matmuls, then band + off-band scores
                                    no = (tt - sb * 4) * TT
                                    bd = slice(no, no + TT)
                                    nc.tensor.matmul(
                                        sc[:, 0, bd], id_sb[:], bn_sb[:],
                                        start=True, stop=False,
                                    )
                                    nc.tensor.matmul(
                                        sc[:, 1, bd], id_sb[:], bn_sb[:],
                                        start=True, stop=False,
                                    )
                                    bs = slice(sb * SB + no, sb * SB + no + TT)
                                    nc.tensor.matmul(
                                        sc[:, 0, bd],
                                        KT[0:64, hp, tts], QT[0:64, hp, bs],
                                        start=False, stop=True,
                                    )
                                    nc.tensor.matmul(
                                        sc[:, 1, bd],
                                        KT[64:128, hp, tts], QT[64:128, hp, bs],
                                        start=False, stop=True,
                                    )
                                    if no + TT < SB:
                                        ms = slice(no + TT, SB)
                                        qs = slice(
                                            sb * SB + no + TT, (sb + 1) * SB
                                        )
                                        nc.tensor.matmul(
                                            sc[:, 0, ms],
                                            KT[0:64, hp, tts], QT[0:64, hp, qs],
                                            start=True, stop=True,
                                        )
                                        nc.tensor.matmul(
                                            sc[:, 1, ms],
                                            KT[64:128, hp, tts],
                                            QT[64:128, hp, qs],
                                            start=True, stop=True,
                                        )
                                else:
                                    no = 0
                                    nc.tensor.matmul(
                                        sc[:, 0, :],
                                        KT[0:64, hp, tts], QT[0:64, hp, sbs],
                                        start=True, stop=True,
                                    )
                                    nc.tensor.matmul(
                                        sc[:, 1, :],
                                        KT[64:128, hp, tts], QT[64:128, hp, sbs],
                                        start=True, stop=True,
                                    )
                                # exp: one strided Act instr per tile (both
                                # heads), narrowed past fully-masked cols
                                et = expp.tile([128, 2, SB], f16, tag="et")
                                nc.scalar.activation(
                                    et[:, :, no:SB], sc[:, :, no:SB],
                                    EXP, scale=0.125,
                                )
                                pend.append((tt, et, no))
                                emit_fillers()
                                if len(pend) > 3:
                                    ptt, pet, pno = pend.pop(0)
                                    nc.tensor.matmul(
                                        av0[0:65, pno:SB],
                                        Vn[:, ptt, 2 * hp, :],
                                        pet[:, 0, pno:SB],
                                        start=(ptt == 0), stop=False,
                                    )
                                    nc.tensor.matmul(
                                        av1[0:65, pno:SB],
                                        Vn[:, ptt, 2 * hp + 1, :],
                                        pet[:, 1, pno:SB],
                                        start=(ptt == 0), stop=False,
                                    )
                            # drain av0 fully first, then av1: av0's softmax
                            # denominator is final half a drain earlier, so
                            # its reciprocal (DVE) runs under av1's matmuls
                            # and the K=1 broadcast matmuls never stall on it
                            rem = list(pend)
                            pend.clear()
                            for bi, av in ((0, av0), (1, av1)):
                                for j, (ptt, pet, pno) in enumerate(rem):
                                    nc.tensor.matmul(
                                        av[0:65, pno:SB],
                                        Vn[:, ptt, 2 * hp + bi, :],
                                        pet[:, bi, pno:SB],
                                        start=(ptt == 0),
                                        stop=(j == len(rem) - 1),
                                    )
                                rdh = smp.tile([1, SB], f16, tag="rdh")
                                with nc.allow_low_precision(
                                    "1/den fits fp16 (den in [1, 2048])"
                                ):
                                    nc.vector.reciprocal(rdh[:], av[64:65, :])
                                xt_tiles[("rdh", bi)] = rdh
                            # normalize: PT[po:po+64, hp, sbs] = av[0:64]/av[64]
                            for po, av in ((0, av0), (64, av1)):
                                rdh = xt_tiles[("rdh", po // 64)]
                                # broadcast 1/den across 64 partitions via a
                                # K=1 fp16 matmul into the unused upper half
                                nc.tensor.matmul(
                                    av[64:128, :], ones_r[:], rdh[:],
                                    start=True, stop=True,
                                )
                                bcs = smp.tile([64, SB], f32, tag="bcs")
                                nc.vector.tensor_copy(bcs[:], av[64:128, :])
                                nc.vector.tensor_tensor(
                                    PT[po : po + 64, hp, sbs],
                                    av[0:64, :], bcs[:], op=MULT,
                                )

                        # drain leftover fillers (early blocks)
                        while fill_i < len(fillers):
                            fn, a = fillers[fill_i]
                            fn(*a)
                            fill_i += 1

                    # ---- epilogue: output projection of the last block,
                    # pipelined across two PSUM banks (ps_pp is done with
                    # projections) and two copy engines (Act is done with
                    # exps) so PE/copy/DMA fully overlap. ----
                    ep = [
                        (st, eb)
                        for st in range(4 * (NSB - 1), 4 * NSB)
                        for eb in range(E // SB)
                    ]
                    for i, (st, eb) in enumerate(ep):
                        oproj_chunk(
                            st, eb,
                            pool=(ps_o, ps_pp)[i % 2],
                            eng=("vector", "scalar")[i % 2],
                            dma="sync",
                        )

    _split_multi_waits(nc)
    return nc


def _get_nc():
    if "nc" not in _cache:
        _cache["nc"] = _build()
    return _cache["nc"]


def _make_runner(nc, n_cores=8):
    """Build a cached jitted SPMD executor (jit once; warm calls are cheap)."""
    import jax
    from jax.sharding import Mesh, PartitionSpec
    from jax.experimental.shard_map import shard_map

    from concourse import mybir
    from concourse.bass2jax import (
        _bass_exec_p,
        install_neuronx_cc_hook,
        partition_id_tensor,
    )

    install_neuronx_cc_hook()
    partition_name = nc.partition_id_tensor.name if nc.partition_id_tensor else None
    in_names, out_names, out_avals, zero_outs = [], [], [], []
    for alloc in nc.m.functions[0].allocations:
        if not isinstance(alloc, mybir.MemoryLocationSet):
            continue
        name = alloc.memorylocations[0].name
        if alloc.kind == "ExternalInput":
            if name != partition_name:
                in_names.append(name)
        elif alloc.kind == "ExternalOutput":
            shape = tuple(alloc.tensor_shape)
            dtype = mybir.dt.np(alloc.dtype)
            out_names.append(name)
            out_avals.append(jax.core.ShapedArray(shape, dtype))
            zero_outs.append(np.zeros(shape, dtype))
    n_params = len(in_names)
    all_in_names = list(in_names) + list(out_names)
    if partition_name is not None:
        all_in_names.append(partition_name)

    def _body(*args):
        operands = list(args)
        if partition_name is not None:
            operands.append(partition_id_tensor())
        return tuple(
            _bass_exec_p.bind(
                *operands,
                out_avals=tuple(out_avals),
                in_names=tuple(all_in_names),
                out_names=tuple(out_names),
                lowering_input_output_aliases=(),
                sim_require_finite=True,
                sim_require_nnan=True,
                nc=nc,
            )
        )

    devices = jax.devices()[:n_cores]
    mesh = Mesh(np.asarray(devices), ("core",))
    in_specs = (PartitionSpec("core"),) * (n_params + len(out_names))
    out_specs = (PartitionSpec("core"),) * len(out_names)
    fn = jax.jit(
        shard_map(
            _body, mesh=mesh, in_specs=in_specs, out_specs=out_specs, check_rep=False
        ),
        keep_unused=True,
    )

    def run(in_maps):
        arrs = [
            np.concatenate([np.asarray(m[name]) for m in in_maps], axis=0)
            for name in in_names
        ]
        zeros = [
            np.zeros((n_cores * z.shape[0], *z.shape[1:]), z.dtype)
            for z in zero_outs
        ]
        outs = fn(*arrs, *zeros)
        per_core = []
        for c in range(n_cores):
            d = {}
            for i, name in enumerate(out_names):
                full = np.asarray(outs[i])
                d[name] = full.reshape(n_cores, full.shape[0] // n_cores, *full.shape[1:])[c]
            per_core.append(d)
        return per_core

    return run


def _get_runner():
    if "run" not in _cache:
        _cache["run"] = _make_runner(_get_nc())
    return _cache["run"]


def _host_inputs(q, k, v, Wq, Wk, Wv, Wo):
    q = np.asarray(q, dtype=np.float32)
    k = np.asarray(k, dtype=np.float32)
    v = np.asarray(v, dtype=np.float32)
    WoT = np.asarray(Wo, dtype=np.float32).T

    qT = [q[b].T.astype(np.float16) for b in range(B)]
    kTb = [k[b].T.astype(np.float16) for b in range(B)]
    vTb = [v[b].T.astype(np.float16) for b in range(B)]

    wqT = [np.asarray(Wq, np.float32)[g * L : (g + 1) * L, :].T.astype(np.float16)
           for g in range(4)]
    wkT = [np.asarray(Wk, np.float32)[g * L : (g + 1) * L, :].T.astype(np.float16)
           for g in range(4)]
    wvT = [np.asarray(Wv, np.float32)[g * L : (g + 1) * L, :].T.astype(np.float16)
           for g in range(4)]
    woT = [WoT[g * L : (g + 1) * L, :].astype(np.float16) for g in range(4)]

    ti = np.arange(128)[:, None]
    cj = np.arange(128)[None, :]
    ident = np.eye(128, dtype=np.float16)
    bnb = np.where(ti > cj, np.float16(-30000.0), np.float16(0.0))

    in_maps = []
    for c in range(8):
        b, g = c // 4, c % 4
        in_maps.append(
            {
                "qT": qT[b], "kT": kTb[b], "vT": vTb[b],
                "wq": wqT[g], "wk": wkT[g], "wv": wvT[g],
                "wo": woT[g], "ident": ident, "bnb": bnb,
            }
        )
    return in_maps


def kernel(q, k, v, Wq, Wk, Wv, Wo, bo):
    run = _get_runner()
    in_maps = _host_inputs(q, k, v, Wq, Wk, Wv, Wo)
    res = run(in_maps)
    out = np.empty((B, S, E), dtype=np.float32)
    bo = np.asarray(bo, dtype=np.float32)
    for b in range(B):
        acc = res[4 * b]["out"].astype(np.float32)
        for g in range(1, 4):
            acc = acc + res[4 * b + g]["out"].astype(np.float32)
        out[b] = acc + bo[None, :]
    return out

